# revision 1
# baseline (speedup 1.0000x reference)
"""Trainium2 kernel for nn_Classification_10651518894899.

M[i, j] = -mean((clip1[j] - clip2[i])**2) * 1e13, then diagonal means.
Expansion: mean((a-b)^2) = m1[j] + m2[i] - 2*cross[i, j] with
  m1[j]      = sum(clip1[j]^2) / F
  m2[i]      = sum(clip2[i]^2) / F
  cross[i,j] = sum(clip2[i] * clip1[j]) / F

Sharding: the flattened pixel dim F = 2764800 is split into 8 contiguous
slabs of 345600 pixels, one per NeuronCore. Each core views its slab
p-major as [128, 2700] (partition p holds pixels p*2700 .. p*2700+2699 of
the slab) so that:
  - HBM->SBUF DMA is 128 partitions x contiguous 1080B runs (fast), and
  - the PE contraction (K=128) runs over partitions with NO transpose:
    gram[i, j] += sum_p c2[p, :, l][i] * c1[p, :, l][j], accumulated over
    all 2700 l-columns in PSUM.
Per-frame sums of squares are computed with ACT Square + DVE segmented
reduce into per-partition partials. The host sums the per-core [20,20]
gram partials and [128,40] norm partials and computes the 21 diagonal
means (tiny).
"""

import numpy as np

N = 20                      # frames per clip
FRAME = 3 * 720 * 1280      # 2764800 pixels per frame
N_CORES = 8
F_CORE = FRAME // N_CORES   # 345600
P = 128
L = F_CORE // P             # 2700
LC = 270                    # l-chunk size
NCHUNK = L // LC            # 10
SCALE = 1e13

_CACHE = {}


def _build_program():
    import concourse.tile as tile
    from concourse import bacc, mybir

    nc = bacc.Bacc("TRN2", target_bir_lowering=False, debug=False)
    c1 = nc.dram_tensor("c1", [N, P, L], mybir.dt.float32, kind="ExternalInput")
    c2 = nc.dram_tensor("c2", [N, P, L], mybir.dt.float32, kind="ExternalInput")
    gram_d = nc.dram_tensor("gram", [N, N], mybir.dt.float32, kind="ExternalOutput")
    nrm_d = nc.dram_tensor("nrm", [P, 2 * N], mybir.dt.float32, kind="ExternalOutput")

    f32 = mybir.dt.float32
    with tile.TileContext(nc) as tc:
        with (
            tc.tile_pool(name="a", bufs=3) as a_pool,
            tc.tile_pool(name="b", bufs=3) as b_pool,
            tc.tile_pool(name="sq", bufs=2) as sq_pool,
            tc.tile_pool(name="misc", bufs=1) as misc,
            tc.tile_pool(name="psum", bufs=1, space="PSUM") as psum_pool,
        ):
            stats = misc.tile([P, 2 * N, NCHUNK], f32)
            pg = psum_pool.tile([N, N], f32)

            for c in range(NCHUNK):
                ls = c * LC
                a_t = a_pool.tile([P, N, LC], f32, tag="a")
                nc.sync.dma_start(
                    out=a_t, in_=c1[:, :, ls : ls + LC].rearrange("j p l -> p j l")
                )
                b_t = b_pool.tile([P, N, LC], f32, tag="b")
                nc.sync.dma_start(
                    out=b_t, in_=c2[:, :, ls : ls + LC].rearrange("j p l -> p j l")
                )

                # cross-gram: gram[i, j] += sum_p c2[p, i, l] * c1[p, j, l]
                for l in range(LC):
                    nc.tensor.matmul(
                        pg,
                        b_t[:, :, l],   # lhsT [K=128, M=20] (c2, stationary)
                        a_t[:, :, l],   # rhs  [K=128, N=20] (c1, moving)
                        start=(c == 0 and l == 0),
                        stop=(c == NCHUNK - 1 and l == LC - 1),
                    )

                # per-frame, per-partition sums of squares
                sq_a = sq_pool.tile([P, N, LC], f32, tag="sq")
                nc.scalar.square(sq_a, a_t)
                nc.vector.tensor_reduce(
                    stats[:, 0:N, c], sq_a,
                    axis=mybir.AxisListType.X, op=mybir.AluOpType.add,
                )
                sq_b = sq_pool.tile([P, N, LC], f32, tag="sq")
                nc.scalar.square(sq_b, b_t)
                nc.vector.tensor_reduce(
                    stats[:, N : 2 * N, c], sq_b,
                    axis=mybir.AxisListType.X, op=mybir.AluOpType.add,
                )

            gram_sb = misc.tile([N, N], f32)
            nc.vector.tensor_copy(gram_sb, pg)
            nrm_sb = misc.tile([P, 2 * N], f32)
            nc.vector.tensor_reduce(
                nrm_sb, stats, axis=mybir.AxisListType.X, op=mybir.AluOpType.add
            )
            nc.sync.dma_start(out=gram_d[:, :], in_=gram_sb)
            nc.sync.dma_start(out=nrm_d[:, :], in_=nrm_sb)

    nc.compile()
    return nc


def _get_program():
    if "nc" not in _CACHE:
        _CACHE["nc"] = _build_program()
    return _CACHE["nc"]


def _run_device(c1_full, c2_full, trace=False, trace_cores=None):
    """c1_full/c2_full: np.float32 [N, FRAME]. Returns (results, bass_results)."""
    from concourse.bass_utils import run_bass_kernel_spmd

    nc = _get_program()
    in_maps = []
    for s in range(N_CORES):
        sl = slice(s * F_CORE, (s + 1) * F_CORE)
        in_maps.append(
            {
                "c1": np.ascontiguousarray(c1_full[:, sl]).reshape(N, P, L),
                "c2": np.ascontiguousarray(c2_full[:, sl]).reshape(N, P, L),
            }
        )
    kwargs = {}
    if trace:
        kwargs["trace"] = True
        if trace_cores is not None:
            kwargs["trace_cores"] = trace_cores
    res = run_bass_kernel_spmd(nc, in_maps, core_ids=list(range(N_CORES)), **kwargs)
    return res


def _postprocess(results):
    gram = np.zeros((N, N), dtype=np.float64)
    nrm = np.zeros(2 * N, dtype=np.float64)
    for r in results:
        gram += r["gram"].astype(np.float64)
        nrm += r["nrm"].astype(np.float64).sum(axis=0)
    f = float(FRAME)
    cross = gram / f            # cross[i, j] = mean(clip2_i * clip1_j)
    m1 = nrm[0:N] / f           # mean(clip1_j ^ 2)
    m2 = nrm[N : 2 * N] / f     # mean(clip2_i ^ 2)
    M = -(m2[:, None] + m1[None, :] - 2.0 * cross) * SCALE
    half = N // 2
    diags = [np.mean(np.diagonal(M, offset=k)) for k in range(-half, half + 1)]
    return np.stack(diags).astype(np.float32)


def kernel(clip1, clip2):
    c1 = np.asarray(clip1, dtype=np.float32).reshape(N, FRAME)
    c2 = np.asarray(clip2, dtype=np.float32).reshape(N, FRAME)
    res = _run_device(c1, c2)
    return _postprocess(res.results)


# revision 5
# speedup vs baseline: 1.0302x; 1.0302x over previous
"""Trainium2 kernel for nn_Classification_10651518894899.

M[i, j] = -mean((clip1[j] - clip2[i])**2) * 1e13, then diagonal means.
Expansion: mean((a-b)^2) = m1[j] + m2[i] - 2*cross[i, j] with
  m1[j]      = sum(clip1[j]^2) / F
  m2[i]      = sum(clip2[i]^2) / F
  cross[i,j] = sum(clip2[i] * clip1[j]) / F

Sharding: the flattened pixel dim F = 2764800 is split into 8 contiguous
slabs of 345600 pixels, one per NeuronCore. Each core views its slab
p-major as [128, 2700] (partition p holds pixels p*2700 .. p*2700+2699 of
the slab) so that:
  - HBM->SBUF DMA is 128 partitions x contiguous 1080B runs (fast), and
  - the PE contraction (K=128) runs over partitions with NO transpose:
    gram[i, j] += sum_p c2[p, :, l][i] * c1[p, :, l][j], accumulated over
    all 2700 l-columns in PSUM.
Per-frame sums of squares are computed with ACT Square + DVE segmented
reduce into per-partition partials. The host sums the per-core [20,20]
gram partials and [128,40] norm partials and computes the 21 diagonal
means (tiny).
"""

import numpy as np

N = 20                      # frames per clip
FRAME = 3 * 720 * 1280      # 2764800 pixels per frame
N_CORES = 8
F_CORE = FRAME // N_CORES   # 345600
P = 128
L = F_CORE // P             # 2700
LC = 270                    # l-chunk size
NCHUNK = L // LC            # 10
SCALE = 1e13

_CACHE = {}


def _build_program():
    import concourse.tile as tile
    from concourse import bacc, mybir

    nc = bacc.Bacc("TRN2", target_bir_lowering=False, debug=False)
    c1 = nc.dram_tensor("c1", [N, P, L], mybir.dt.float32, kind="ExternalInput")
    c2 = nc.dram_tensor("c2", [N, P, L], mybir.dt.float32, kind="ExternalInput")
    gram_d = nc.dram_tensor("gram", [P, N], mybir.dt.float32, kind="ExternalOutput")
    nrm_d = nc.dram_tensor("nrm", [P, 2 * N], mybir.dt.float32, kind="ExternalOutput")

    f32 = mybir.dt.float32
    with tile.TileContext(nc) as tc:
        with (
            tc.tile_pool(name="a", bufs=3) as a_pool,
            tc.tile_pool(name="b", bufs=3) as b_pool,
            tc.tile_pool(name="sq", bufs=2) as sq_pool,
            tc.tile_pool(name="misc", bufs=1) as misc,
            tc.tile_pool(name="psum", bufs=1, space="PSUM") as psum_pool,
        ):
            stats = misc.tile([P, 2 * N, NCHUNK], f32)
            # 4 independent accumulators, one per 32-column PE array group
            # (col-tiling: l-column ℓ goes to group ℓ % 4). Host sums them.
            pg = psum_pool.tile([P, N], f32)

            for c in range(NCHUNK):
                ls = c * LC
                a_t = a_pool.tile([P, N, LC], f32, tag="a")
                nc.sync.dma_start(
                    out=a_t, in_=c1[:, :, ls : ls + LC].rearrange("j p l -> p j l")
                )
                b_t = b_pool.tile([P, N, LC], f32, tag="b")
                nc.sync.dma_start(
                    out=b_t, in_=c2[:, :, ls : ls + LC].rearrange("j p l -> p j l")
                )

                # cross-gram: gram[i, j] += sum_p c2[p, i, l] * c1[p, j, l]
                for l in range(LC):
                    lg = ls + l          # global l index in [0, L)
                    g = lg % 4           # PE column group
                    nc.tensor.matmul(
                        pg[32 * g : 32 * g + N, :],
                        b_t[:, :, l],   # lhsT [K=128, M=20] (c2, stationary)
                        a_t[:, :, l],   # rhs  [K=128, N=20] (c1, moving)
                        start=(lg == g),
                        stop=(lg == L - 4 + g),
                        tile_position=(0, 32 * g),
                    )

                # per-frame, per-partition sums of squares
                sq_a = sq_pool.tile([P, N, LC], f32, tag="sq")
                nc.scalar.square(sq_a, a_t)
                nc.vector.tensor_reduce(
                    stats[:, 0:N, c], sq_a,
                    axis=mybir.AxisListType.X, op=mybir.AluOpType.add,
                )
                sq_b = sq_pool.tile([P, N, LC], f32, tag="sq")
                nc.scalar.square(sq_b, b_t)
                nc.vector.tensor_reduce(
                    stats[:, N : 2 * N, c], sq_b,
                    axis=mybir.AxisListType.X, op=mybir.AluOpType.add,
                )

            gram_sb = misc.tile([P, N], f32)
            nc.vector.tensor_copy(gram_sb, pg)
            nrm_sb = misc.tile([P, 2 * N], f32)
            nc.vector.tensor_reduce(
                nrm_sb, stats, axis=mybir.AxisListType.X, op=mybir.AluOpType.add
            )
            nc.sync.dma_start(out=gram_d[:, :], in_=gram_sb)
            nc.sync.dma_start(out=nrm_d[:, :], in_=nrm_sb)

    nc.compile()
    return nc


def _get_program():
    if "nc" not in _CACHE:
        _CACHE["nc"] = _build_program()
    return _CACHE["nc"]


def _run_device(c1_full, c2_full, trace=False, trace_cores=None):
    """c1_full/c2_full: np.float32 [N, FRAME]. Returns (results, bass_results)."""
    from concourse.bass_utils import run_bass_kernel_spmd

    nc = _get_program()
    in_maps = []
    for s in range(N_CORES):
        sl = slice(s * F_CORE, (s + 1) * F_CORE)
        in_maps.append(
            {
                "c1": np.ascontiguousarray(c1_full[:, sl]).reshape(N, P, L),
                "c2": np.ascontiguousarray(c2_full[:, sl]).reshape(N, P, L),
            }
        )
    kwargs = {}
    if trace:
        kwargs["trace"] = True
        if trace_cores is not None:
            kwargs["trace_cores"] = trace_cores
    res = run_bass_kernel_spmd(nc, in_maps, core_ids=list(range(N_CORES)), **kwargs)
    return res


def _postprocess(results):
    gram = np.zeros((N, N), dtype=np.float64)
    nrm = np.zeros(2 * N, dtype=np.float64)
    for r in results:
        g = r["gram"].astype(np.float64)
        for j in range(4):
            gram += g[32 * j : 32 * j + N]
        nrm += r["nrm"].astype(np.float64).sum(axis=0)
    f = float(FRAME)
    cross = gram / f            # cross[i, j] = mean(clip2_i * clip1_j)
    m1 = nrm[0:N] / f           # mean(clip1_j ^ 2)
    m2 = nrm[N : 2 * N] / f     # mean(clip2_i ^ 2)
    M = -(m2[:, None] + m1[None, :] - 2.0 * cross) * SCALE
    half = N // 2
    diags = [np.mean(np.diagonal(M, offset=k)) for k in range(-half, half + 1)]
    return np.stack(diags).astype(np.float32)


def kernel(clip1, clip2):
    c1 = np.asarray(clip1, dtype=np.float32).reshape(N, FRAME)
    c2 = np.asarray(clip2, dtype=np.float32).reshape(N, FRAME)
    res = _run_device(c1, c2)
    return _postprocess(res.results)


# revision 8
# speedup vs baseline: 1.0806x; 1.0489x over previous
"""Trainium2 kernel for nn_Classification_10651518894899.

M[i, j] = -mean((clip1[j] - clip2[i])**2) * 1e13, then diagonal means.
Expansion: mean((a-b)^2) = m1[j] + m2[i] - 2*cross[i, j] with
  m1[j]      = sum(clip1[j]^2) / F
  m2[i]      = sum(clip2[i]^2) / F
  cross[i,j] = sum(clip2[i] * clip1[j]) / F

Sharding: the flattened pixel dim F = 2764800 is split into 8 contiguous
slabs of 345600 pixels, one per NeuronCore. Each core views its slab
p-major as [128, 2700] (partition p holds pixels p*2700 .. p*2700+2699 of
the slab) so that:
  - HBM->SBUF DMA is 128 partitions x contiguous 1080B runs (fast), and
  - the PE contraction (K=128) runs over partitions with NO transpose:
    gram[i, j] += sum_p c2[p, :, l][i] * c1[p, :, l][j], accumulated over
    all 2700 l-columns in PSUM.
Per-frame sums of squares are computed with ACT Square + DVE segmented
reduce into per-partition partials. The host sums the per-core [20,20]
gram partials and [128,40] norm partials and computes the 21 diagonal
means (tiny).
"""

import numpy as np

N = 20                      # frames per clip
FRAME = 3 * 720 * 1280      # 2764800 pixels per frame
N_CORES = 8
F_CORE = FRAME // N_CORES   # 345600
P = 128
L = F_CORE // P             # 2700
LC = 270                    # l-chunk size
NCHUNK = L // LC            # 10
SCALE = 1e13

_CACHE = {}


def _build_program():
    import concourse.tile as tile
    from concourse import bacc, mybir

    nc = bacc.Bacc("TRN2", target_bir_lowering=False, debug=False)
    # host pre-arranges each core's slab as [chunk, p, frame, l] so every
    # chunk DMA is one fully contiguous HBM block (21.6KB/partition runs)
    c1 = nc.dram_tensor("c1", [NCHUNK, P, N, LC], mybir.dt.float32, kind="ExternalInput")
    c2 = nc.dram_tensor("c2", [NCHUNK, P, N, LC], mybir.dt.float32, kind="ExternalInput")
    gram_d = nc.dram_tensor("gram", [P, N], mybir.dt.float32, kind="ExternalOutput")
    nrm_d = nc.dram_tensor("nrm", [P, 2 * N], mybir.dt.float32, kind="ExternalOutput")

    f32 = mybir.dt.float32
    with tile.TileContext(nc) as tc:
        with (
            tc.tile_pool(name="a", bufs=3) as a_pool,
            tc.tile_pool(name="b", bufs=3) as b_pool,
            tc.tile_pool(name="sq", bufs=2) as sq_pool,
            tc.tile_pool(name="misc", bufs=1) as misc,
            tc.tile_pool(name="psum", bufs=1, space="PSUM") as psum_pool,
        ):
            stats = misc.tile([P, 2 * N, NCHUNK], f32)
            # 4 independent accumulators, one per 32-column PE array group
            # (col-tiling: l-column ℓ goes to group ℓ % 4). Host sums them.
            pg = psum_pool.tile([P, N], f32)

            for c in range(NCHUNK):
                ls = c * LC
                a_t = a_pool.tile([P, N, LC], f32, tag="a")
                nc.sync.dma_start(out=a_t, in_=c1[c])
                b_t = b_pool.tile([P, N, LC], f32, tag="b")
                nc.sync.dma_start(out=b_t, in_=c2[c])

                # cross-gram: gram[i, j] += sum_p c2[p, i, l] * c1[p, j, l]
                for l in range(LC):
                    lg = ls + l          # global l index in [0, L)
                    g = lg % 4           # PE column group
                    nc.tensor.matmul(
                        pg[32 * g : 32 * g + N, :],
                        b_t[:, :, l],   # lhsT [K=128, M=20] (c2, stationary)
                        a_t[:, :, l],   # rhs  [K=128, N=20] (c1, moving)
                        start=(lg == g),
                        stop=(lg == L - 4 + g),
                        tile_position=(0, 32 * g),
                    )

                # per-frame, per-partition sums of squares
                sq_a = sq_pool.tile([P, N, LC], f32, tag="sq")
                nc.scalar.square(sq_a, a_t)
                nc.vector.tensor_reduce(
                    stats[:, 0:N, c], sq_a,
                    axis=mybir.AxisListType.X, op=mybir.AluOpType.add,
                )
                sq_b = sq_pool.tile([P, N, LC], f32, tag="sq")
                nc.scalar.square(sq_b, b_t)
                nc.vector.tensor_reduce(
                    stats[:, N : 2 * N, c], sq_b,
                    axis=mybir.AxisListType.X, op=mybir.AluOpType.add,
                )

            gram_sb = misc.tile([P, N], f32)
            nc.vector.tensor_copy(gram_sb, pg)
            nrm_sb = misc.tile([P, 2 * N], f32)
            nc.vector.tensor_reduce(
                nrm_sb, stats, axis=mybir.AxisListType.X, op=mybir.AluOpType.add
            )
            nc.sync.dma_start(out=gram_d[:, :], in_=gram_sb)
            nc.sync.dma_start(out=nrm_d[:, :], in_=nrm_sb)

    nc.compile()
    return nc


def _get_program():
    if "nc" not in _CACHE:
        _CACHE["nc"] = _build_program()
    return _CACHE["nc"]


def _run_device(c1_full, c2_full, trace=False, trace_cores=None):
    """c1_full/c2_full: np.float32 [N, FRAME]. Returns (results, bass_results)."""
    from concourse.bass_utils import run_bass_kernel_spmd

    nc = _get_program()

    def shard(full, s):
        # slab [N, F_CORE] -> [N, P, NCHUNK, LC] -> [NCHUNK, P, N, LC]
        slab = full[:, s * F_CORE : (s + 1) * F_CORE]
        return np.ascontiguousarray(
            slab.reshape(N, P, NCHUNK, LC).transpose(2, 1, 0, 3)
        )

    in_maps = []
    for s in range(N_CORES):
        in_maps.append({"c1": shard(c1_full, s), "c2": shard(c2_full, s)})
    kwargs = {}
    if trace:
        kwargs["trace"] = True
        if trace_cores is not None:
            kwargs["trace_cores"] = trace_cores
    res = run_bass_kernel_spmd(nc, in_maps, core_ids=list(range(N_CORES)), **kwargs)
    return res


def _postprocess(results):
    gram = np.zeros((N, N), dtype=np.float64)
    nrm = np.zeros(2 * N, dtype=np.float64)
    for r in results:
        g = r["gram"].astype(np.float64)
        for j in range(4):
            gram += g[32 * j : 32 * j + N]
        nrm += r["nrm"].astype(np.float64).sum(axis=0)
    f = float(FRAME)
    cross = gram / f            # cross[i, j] = mean(clip2_i * clip1_j)
    m1 = nrm[0:N] / f           # mean(clip1_j ^ 2)
    m2 = nrm[N : 2 * N] / f     # mean(clip2_i ^ 2)
    M = -(m2[:, None] + m1[None, :] - 2.0 * cross) * SCALE
    half = N // 2
    diags = [np.mean(np.diagonal(M, offset=k)) for k in range(-half, half + 1)]
    return np.stack(diags).astype(np.float32)


def kernel(clip1, clip2):
    c1 = np.asarray(clip1, dtype=np.float32).reshape(N, FRAME)
    c2 = np.asarray(clip2, dtype=np.float32).reshape(N, FRAME)
    res = _run_device(c1, c2)
    return _postprocess(res.results)
